# revision 20
# baseline (speedup 1.0000x reference)
"""AveragePrevEmbeddingsLM Trainium2 kernel (8 NeuronCores, vocab-sharded).

logits[b, t, v] = mean(emb_table[x[b, :t+1]]) @ W.T + b_vec

Strategy: shard the vocab dim across 8 cores (4000 each). Every core
redundantly gathers + prefix-sums all 8192 token embeddings (cheap),
then computes its (8192 x 64) @ (64 x 4000) logits slice. The logits
write is the memory roofline (~65.5 MB/core in f16).

Device pipeline per core:
  dma_gather (emb rows, per batch)  -> [128tok, 16blk, 64emb] SBUF
  PE transpose per 128-token block  -> [64emb, 128tok] PSUM -> SBUF seg
  tensor_tensor_scan along seq      -> causal prefix sums (f32)
  cast to bf16 + DMA-dup rows 0-63 -> partitions 64-127
  per m-tile PAIR: 2x K=64 row-tiled matmuls run concurrently in the
  128x128 PE array (tile_position (0,0) / (64,0)); even m-tile's seg
  block streams from partitions 0-63, odd from 64-127, against a
  host-duplicated W.T (rows 64-127 = rows 0-63). Halves PE wall time
  so even a HAM-cold PE (1.2 GHz) stays under the DMA cadence.
  ACT/DVE drain PSUM with fused 1000-col scaled copies (x 1/(pos+1))
  -> f16 SBUF -> 512KB DMAs out.

The bias is NOT applied on device (it would force K=65 and kill row
tiling); the host adds b to the assembled f32 output. Output is
written f16 (rel err ~5e-4 << 2e-2 gate) halving HBM write traffic.
"""

import os
import sys

import numpy as np

for _p in ("/opt/trn_rl_repo",):
    if _p not in sys.path and os.path.isdir(_p):
        sys.path.append(_p)

VOCAB, EMB, B, SEQ = 32000, 64, 4, 2048
NCORES = 8
VS = VOCAB // NCORES       # vocab shard per core
TOK = B * SEQ
BLK = SEQ // 128           # 128-token blocks per batch row
MTILES = TOK // 128
NCHUNK = 8
CHUNK = VS // NCHUNK       # matmul free-dim chunk (500)

COMPUTE = os.environ.get("KERNEL_COMPUTE", "bf16")   # bf16 | f16 | f32r
OUT_DT = os.environ.get("KERNEL_OUT_DT", "f16")      # f16 | bf16 | f32

_prog_cache = {}


def _build(compute: str, out_dt: str):
    from concourse import bacc
    import concourse.mybir as mybir
    import concourse.tile as tile
    from concourse.masks import make_identity

    f32 = mybir.dt.float32
    cdt = {
        "bf16": mybir.dt.bfloat16,
        "f16": mybir.dt.float16,
        "f32r": mybir.dt.float32r,
    }[compute]
    odt = {
        "f16": mybir.dt.float16,
        "bf16": mybir.dt.bfloat16,
        "f32": f32,
    }[out_dt]

    nc = bacc.Bacc(None, target_bir_lowering=False)

    emb_d = nc.dram_tensor("emb", [VOCAB, EMB], f32, kind="ExternalInput")
    idx_d = nc.dram_tensor("idx", [128, MTILES], mybir.dt.int32, kind="ExternalInput")
    wtb_d = nc.dram_tensor("wtb", [128, VS], f32, kind="ExternalInput")
    recip_d = nc.dram_tensor("recip", [128, BLK], f32, kind="ExternalInput")
    out_d = nc.dram_tensor("out", [TOK, VS], odt, kind="ExternalOutput")

    with tile.TileContext(nc) as tc:
        with (
            tc.tile_pool(name="const", bufs=1) as constp,
            tc.tile_pool(name="gath", bufs=2) as gathp,
            tc.tile_pool(name="segraw", bufs=2) as segrawp,
            tc.tile_pool(name="segcum", bufs=2) as segcump,
            tc.tile_pool(name="outp", bufs=4) as outp,
            tc.tile_pool(name="ptr", bufs=1, space="PSUM") as ptrp,
            tc.tile_pool(name="pmm", bufs=3, space="PSUM") as pmmp,
            tc.tile_pool(name="warm", bufs=1, space="PSUM") as warmp,
        ):
            wtb_sb = constp.tile([128, VS], f32)
            nc.sync.dma_start(wtb_sb[:], wtb_d[:])
            recip_sb = constp.tile([128, BLK], f32)
            nc.sync.dma_start(recip_sb[:], recip_d[:])
            idx_sb = constp.tile([128, MTILES], mybir.dt.int32)
            nc.sync.dma_start(idx_sb[:], idx_d[:])
            ident = constp.tile([128, 128], f32)
            make_identity(nc, ident[:])

            # W.T (duplicated in rows 64-127 by the host) cast once
            wtb_c = constp.tile([128, VS], cdt)
            nc.vector.tensor_copy(wtb_c[:], wtb_sb[:])

            import concourse.bass as bass

            # HAM keepalive: the DMA-paced pipeline leaves sub-µs PE
            # idle gaps every chunk-group, which keeps the PE clock
            # throttled at 1.2 GHz (cold) — and a cold PE is slower
            # than the DMA roofline. Dummy matmuls into a write-only
            # scratch bank fill the gaps so the PE stays warm (2.4
            # GHz). Nothing ever reads the bank: no drain, no
            # collision (PE-write only).
            warm_sb = warmp.tile([128, 512], f32)

            def keepalive(n=1):
                for _ in range(n):
                    nc.tensor.matmul(
                        warm_sb[:, 0:512],
                        wtb_c[0:EMB, 0:128],
                        wtb_c[0:EMB, 0:512],
                        start=True, stop=True,
                    )

            # Software pipeline at 512-token (4 m-tile) "quarter"
            # granularity: head(Q) = gather + PE-transpose + chained
            # scan + cast + upper-dup; proj(Q) = 2 m-tile PAIRS of
            # row-tiled matmuls + fused scaled copies + DMA out.
            # head(Q+1) is emitted before proj(Q) so each engine's
            # in-order stream interleaves next-quarter prep with
            # current projections.
            QT = 4                      # m-tiles per quarter
            NQ = MTILES // QT           # total quarters (16)
            QSEQ = QT * 128             # tokens per quarter (512)
            state = {}

            def head(Q):
                b, q = Q // (BLK // QT), Q % (BLK // QT)
                if q == 0:
                    state["gath"] = gathp.tile(
                        [128, BLK, EMB], f32, tag="gath", name="gath")
                    state["seg_raw"] = segrawp.tile(
                        [EMB, SEQ], f32, tag="seg_raw", name="seg_raw")
                    state["seg_cum"] = segcump.tile(
                        [EMB, SEQ], f32, tag="seg_cum", name="seg_cum")
                    state["seg_pair"] = segcump.tile(
                        [128, SEQ], cdt, tag="seg_pair", name="seg_pair")
                gath, seg_raw = state["gath"], state["seg_raw"]
                seg_cum, seg_pair = state["seg_cum"], state["seg_pair"]
                for mb in range(q * QT, (q + 1) * QT):
                    m = b * BLK + mb
                    nc.gpsimd.indirect_dma_start(
                        out=gath[:, mb, :],
                        out_offset=None,
                        in_=emb_d[:],
                        in_offset=bass.IndirectOffsetOnAxis(
                            ap=idx_sb[:, m:m + 1], axis=0,
                        ),
                    )
                    pt = ptrp.tile([EMB, 128], f32)
                    nc.tensor.transpose(pt[:], gath[:, mb, :], ident[:])
                    nc.scalar.activation(
                        seg_raw[:, mb * 128:(mb + 1) * 128], pt[:],
                        mybir.ActivationFunctionType.Copy,
                    )
                    keepalive(1)
                qsl = slice(q * QSEQ, (q + 1) * QSEQ)
                initial = (0.0 if q == 0 else
                           seg_cum[:, q * QSEQ - 1:q * QSEQ])
                nc.vector.tensor_tensor_scan(
                    seg_cum[:, qsl],
                    seg_raw[:, qsl],
                    seg_raw[:, qsl],
                    initial,
                    op0=mybir.AluOpType.add,
                    op1=mybir.AluOpType.bypass,
                )
                nc.vector.tensor_copy(seg_pair[0:EMB, qsl], seg_cum[:, qsl])
                # duplicate to partitions 64-127 for the odd row-tile
                nc.sync.dma_start(seg_pair[EMB:128, qsl],
                                  seg_pair[0:EMB, qsl])
                state["seg_c"] = seg_pair[:]

            def proj(Q, seg_c):
                b, q = Q // (BLK // QT), Q % (BLK // QT)
                for pi in range(QT // 2):
                    mbA = q * QT + 2 * pi
                    mbB = mbA + 1
                    mA = b * BLK + mbA
                    mB = b * BLK + mbB
                    lhsT_A = seg_c[0:EMB, mbA * 128:(mbA + 1) * 128]
                    lhsT_B = seg_c[EMB:128, mbB * 128:(mbB + 1) * 128]
                    scale_A = recip_sb[:, mbA:mbA + 1]
                    scale_B = recip_sb[:, mbB:mbB + 1]
                    otA = outp.tile([128, NCHUNK, CHUNK], odt,
                                    tag="ot", name="ot")
                    otB = outp.tile([128, NCHUNK, CHUNK], odt,
                                    tag="ot", name="ot")
                    # 4 chunk-groups of 2; each group: 2+2 concurrent
                    # row-tiled matmuls into two 2-bank PSUM tiles,
                    # fused 1000-col scaled copies draining in
                    # PARALLEL (tA always on ACT, tB always on DVE),
                    # 512KB DMA per half.
                    for cg in range(4):
                        tA = pmmp.tile([128, 2, 512], f32,
                                       tag="pmm", name="pmm")
                        tB = pmmp.tile([128, 2, 512], f32,
                                       tag="pmm", name="pmm")
                        for k in range(2):
                            ch = 2 * cg + k
                            csl = slice(ch * CHUNK, (ch + 1) * CHUNK)
                            nc.tensor.matmul(
                                tA[:, k, 0:CHUNK], lhsT_A,
                                wtb_c[0:EMB, csl],
                                start=True, stop=True,
                            )
                            nc.tensor.matmul(
                                tB[:, k, 0:CHUNK], lhsT_B,
                                wtb_c[EMB:128, csl],
                                start=True, stop=True,
                            )
                        keepalive(1)
                        nc.scalar.activation(
                            otA[:, 2 * cg:2 * cg + 2, :], tA[:, :, 0:CHUNK],
                            mybir.ActivationFunctionType.Copy,
                            scale=scale_A,
                        )
                        nc.vector.tensor_scalar_mul(
                            otB[:, 2 * cg:2 * cg + 2, :],
                            tB[:, :, 0:CHUNK], scale_B)
                        if cg == 1:
                            nc.sync.dma_start(
                                out_d[mA * 128:(mA + 1) * 128, 0:VS // 2],
                                otA[:, 0:NCHUNK // 2, :])
                            nc.sync.dma_start(
                                out_d[mB * 128:(mB + 1) * 128, 0:VS // 2],
                                otB[:, 0:NCHUNK // 2, :])
                        elif cg == 3:
                            nc.sync.dma_start(
                                out_d[mA * 128:(mA + 1) * 128, VS // 2:VS],
                                otA[:, NCHUNK // 2:NCHUNK, :])
                            nc.sync.dma_start(
                                out_d[mB * 128:(mB + 1) * 128, VS // 2:VS],
                                otB[:, NCHUNK // 2:NCHUNK, :])

            LEAD = 1
            seg_of = {}
            for Q in range(min(LEAD, NQ)):
                head(Q)
                seg_of[Q] = state["seg_c"]
            for Q in range(NQ):
                if Q + LEAD < NQ:
                    head(Q + LEAD)
                    seg_of[Q + LEAD] = state["seg_c"]
                proj(Q, seg_of.pop(Q))

    nc.compile()
    return nc


def _get_prog(compute: str, out_dt: str):
    key = (compute, out_dt)
    if key not in _prog_cache:
        _prog_cache[key] = _build(compute, out_dt)
    return _prog_cache[key]


def _make_in_maps(emb_table, W, b, x):
    emb_table = np.ascontiguousarray(np.asarray(emb_table, dtype=np.float32))
    W = np.asarray(W, dtype=np.float32)
    x = np.asarray(x).astype(np.int64).reshape(B, SEQ)

    # idx layout: token m*128 + p -> idx[p, m]
    wrapped = np.ascontiguousarray(
        x.reshape(-1).reshape(MTILES, 128).T.astype(np.int32)
    )

    i = np.arange(128)[:, None]
    mb = np.arange(BLK)[None, :]
    recip = (1.0 / (mb * 128 + i + 1)).astype(np.float32)

    in_maps = []
    for c in range(NCORES):
        wtb = np.empty((128, VS), dtype=np.float32)
        wtb[0:EMB] = W[c * VS:(c + 1) * VS, :].T
        wtb[EMB:128] = wtb[0:EMB]
        in_maps.append({
            "emb": emb_table,
            "idx": wrapped,
            "wtb": np.ascontiguousarray(wtb),
            "recip": recip,
        })
    return in_maps


def kernel(emb_table, W, b, x, trace=False):
    from concourse.bass_utils import run_bass_kernel_spmd

    nc = _get_prog(COMPUTE, OUT_DT)
    in_maps = _make_in_maps(emb_table, W, b, x)
    res = run_bass_kernel_spmd(
        nc, in_maps, core_ids=list(range(NCORES)), trace=trace,
    )

    b32 = np.asarray(b, dtype=np.float32)
    out = np.empty((TOK, VOCAB), dtype=np.float32)
    for c in range(NCORES):
        sl = slice(c * VS, (c + 1) * VS)
        np.add(np.asarray(res.results[c]["out"], dtype=np.float32),
               b32[sl][None, :], out=out[:, sl])
    out = out.reshape(B, SEQ, VOCAB)
    if trace:
        return out, res
    return out


# revision 21
# speedup vs baseline: 1.1637x; 1.1637x over previous
"""AveragePrevEmbeddingsLM Trainium2 kernel (8 NeuronCores, vocab-sharded).

logits[b, t, v] = mean(emb_table[x[b, :t+1]]) @ W.T + b_vec

Strategy: shard the vocab dim across 8 cores (4000 each). Every core
redundantly gathers + prefix-sums all 8192 token embeddings (cheap),
then computes its (8192 x 64) @ (64 x 4000) logits slice. The logits
write is the memory roofline (~65.5 MB/core in f16).

Device pipeline per core:
  dma_gather (emb rows, per batch)  -> [128tok, 16blk, 64emb] SBUF
  PE transpose per 128-token block  -> [64emb, 128tok] PSUM -> SBUF seg
  tensor_tensor_scan along seq      -> causal prefix sums (f32)
  cast to bf16 + DMA-dup rows 0-63 -> partitions 64-127
  per m-tile PAIR: 2x K=64 row-tiled matmuls run concurrently in the
  128x128 PE array (tile_position (0,0) / (64,0)); even m-tile's seg
  block streams from partitions 0-63, odd from 64-127, against a
  host-duplicated W.T (rows 64-127 = rows 0-63). Halves PE wall time
  so even a HAM-cold PE (1.2 GHz) stays under the DMA cadence.
  ACT/DVE drain PSUM with fused 1000-col scaled copies (x 1/(pos+1))
  -> f16 SBUF -> 512KB DMAs out.

The bias is NOT applied on device (it would force K=65 and kill row
tiling); the host adds b to the assembled f32 output. Output is
written f16 (rel err ~5e-4 << 2e-2 gate) halving HBM write traffic.
"""

import os
import sys

import numpy as np

for _p in ("/opt/trn_rl_repo",):
    if _p not in sys.path and os.path.isdir(_p):
        sys.path.append(_p)

VOCAB, EMB, B, SEQ = 32000, 64, 4, 2048
NCORES = 8
VS = VOCAB // NCORES       # vocab shard per core
TOK = B * SEQ
BLK = SEQ // 128           # 128-token blocks per batch row
MTILES = TOK // 128
NCHUNK = 8
CHUNK = VS // NCHUNK       # matmul free-dim chunk (500)

COMPUTE = os.environ.get("KERNEL_COMPUTE", "bf16")   # bf16 | f16 | f32r
OUT_DT = os.environ.get("KERNEL_OUT_DT", "f16")      # f16 | bf16 | f32

_prog_cache = {}


def _build(compute: str, out_dt: str):
    from concourse import bacc
    import concourse.mybir as mybir
    import concourse.tile as tile
    from concourse.masks import make_identity

    f32 = mybir.dt.float32
    cdt = {
        "bf16": mybir.dt.bfloat16,
        "f16": mybir.dt.float16,
        "f32r": mybir.dt.float32r,
    }[compute]
    odt = {
        "f16": mybir.dt.float16,
        "bf16": mybir.dt.bfloat16,
        "f32": f32,
    }[out_dt]

    nc = bacc.Bacc(None, target_bir_lowering=False)

    emb_d = nc.dram_tensor("emb", [VOCAB, EMB], f32, kind="ExternalInput")
    idx_d = nc.dram_tensor("idx", [128, MTILES], mybir.dt.int32, kind="ExternalInput")
    wtb_d = nc.dram_tensor("wtb", [128, VS], f32, kind="ExternalInput")
    recip_d = nc.dram_tensor("recip", [128, BLK], f32, kind="ExternalInput")
    out_d = nc.dram_tensor("out", [TOK, VS], odt, kind="ExternalOutput")

    with tile.TileContext(nc) as tc:
        with (
            tc.tile_pool(name="const", bufs=1) as constp,
            tc.tile_pool(name="gath", bufs=2) as gathp,
            tc.tile_pool(name="segraw", bufs=2) as segrawp,
            tc.tile_pool(name="segcum", bufs=2) as segcump,
            tc.tile_pool(name="outp", bufs=4) as outp,
            tc.tile_pool(name="ptr", bufs=1, space="PSUM") as ptrp,
            tc.tile_pool(name="pmm", bufs=3, space="PSUM") as pmmp,
            tc.tile_pool(name="warm", bufs=1, space="PSUM") as warmp,
        ):
            wtb_sb = constp.tile([128, VS], f32)
            nc.sync.dma_start(wtb_sb[:], wtb_d[:])
            recip_sb = constp.tile([128, BLK], f32)
            nc.sync.dma_start(recip_sb[:], recip_d[:])
            idx_sb = constp.tile([128, MTILES], mybir.dt.int32)
            nc.sync.dma_start(idx_sb[:], idx_d[:])
            ident = constp.tile([128, 128], f32)
            make_identity(nc, ident[:])

            # W.T (duplicated in rows 64-127 by the host) cast once
            wtb_c = constp.tile([128, VS], cdt)
            nc.vector.tensor_copy(wtb_c[:], wtb_sb[:])

            import concourse.bass as bass

            # HAM keepalive: the DMA-paced pipeline leaves sub-µs PE
            # idle gaps every chunk-group, which keeps the PE clock
            # throttled at 1.2 GHz (cold) — and a cold PE is slower
            # than the DMA roofline. Dummy matmuls into a write-only
            # scratch bank fill the gaps so the PE stays warm (2.4
            # GHz). Nothing ever reads the bank: no drain, no
            # collision (PE-write only).
            warm_sb = warmp.tile([128, 512], f32)

            def keepalive(n=1):
                for _ in range(n):
                    nc.tensor.matmul(
                        warm_sb[:, 0:512],
                        wtb_c[0:EMB, 0:128],
                        wtb_c[0:EMB, 0:512],
                        start=True, stop=True,
                    )

            # Software pipeline at 512-token (4 m-tile) "quarter"
            # granularity: head(Q) = gather + PE-transpose + chained
            # scan + cast + upper-dup; proj(Q) = 2 m-tile PAIRS of
            # row-tiled matmuls + fused scaled copies + DMA out.
            # head(Q+1) is emitted before proj(Q) so each engine's
            # in-order stream interleaves next-quarter prep with
            # current projections.
            QT = 4                      # m-tiles per quarter
            NQ = MTILES // QT           # total quarters (16)
            QSEQ = QT * 128             # tokens per quarter (512)
            state = {}

            def head(Q):
                b, q = Q // (BLK // QT), Q % (BLK // QT)
                if q == 0:
                    state["gath"] = gathp.tile(
                        [128, BLK, EMB], f32, tag="gath", name="gath")
                    state["seg_raw"] = segrawp.tile(
                        [EMB, SEQ], f32, tag="seg_raw", name="seg_raw")
                    state["seg_cum"] = segcump.tile(
                        [EMB, SEQ], f32, tag="seg_cum", name="seg_cum")
                    state["seg_pair"] = segcump.tile(
                        [128, SEQ], cdt, tag="seg_pair", name="seg_pair")
                gath, seg_raw = state["gath"], state["seg_raw"]
                seg_cum, seg_pair = state["seg_cum"], state["seg_pair"]
                for mb in range(q * QT, (q + 1) * QT):
                    m = b * BLK + mb
                    nc.gpsimd.indirect_dma_start(
                        out=gath[:, mb, :],
                        out_offset=None,
                        in_=emb_d[:],
                        in_offset=bass.IndirectOffsetOnAxis(
                            ap=idx_sb[:, m:m + 1], axis=0,
                        ),
                    )
                    pt = ptrp.tile([EMB, 128], f32)
                    nc.tensor.transpose(pt[:], gath[:, mb, :], ident[:])
                    nc.scalar.activation(
                        seg_raw[:, mb * 128:(mb + 1) * 128], pt[:],
                        mybir.ActivationFunctionType.Copy,
                    )
                    keepalive(1)
                qsl = slice(q * QSEQ, (q + 1) * QSEQ)
                initial = (0.0 if q == 0 else
                           seg_cum[:, q * QSEQ - 1:q * QSEQ])
                nc.vector.tensor_tensor_scan(
                    seg_cum[:, qsl],
                    seg_raw[:, qsl],
                    seg_raw[:, qsl],
                    initial,
                    op0=mybir.AluOpType.add,
                    op1=mybir.AluOpType.bypass,
                )
                nc.vector.tensor_copy(seg_pair[0:EMB, qsl], seg_cum[:, qsl])
                # duplicate to partitions 64-127 for the odd row-tile
                nc.sync.dma_start(seg_pair[EMB:128, qsl],
                                  seg_pair[0:EMB, qsl])
                state["seg_c"] = seg_pair[:]

            def proj(Q, seg_c):
                b, q = Q // (BLK // QT), Q % (BLK // QT)
                for pi in range(QT // 2):
                    mbA = q * QT + 2 * pi
                    mbB = mbA + 1
                    mA = b * BLK + mbA
                    mB = b * BLK + mbB
                    lhsT_A = seg_c[0:EMB, mbA * 128:(mbA + 1) * 128]
                    lhsT_B = seg_c[EMB:128, mbB * 128:(mbB + 1) * 128]
                    scale_A = recip_sb[:, mbA:mbA + 1]
                    scale_B = recip_sb[:, mbB:mbB + 1]
                    otA = outp.tile([128, NCHUNK, CHUNK], odt,
                                    tag="ot", name="ot")
                    otB = outp.tile([128, NCHUNK, CHUNK], odt,
                                    tag="ot", name="ot")
                    # 4 chunk-groups of 2; each group: 2+2 concurrent
                    # row-tiled matmuls into two 2-bank PSUM tiles,
                    # fused 1000-col scaled copies draining in
                    # PARALLEL (tA always on ACT, tB always on DVE),
                    # 512KB DMA per half.
                    for cg in range(4):
                        tA = pmmp.tile([128, 2, 512], f32,
                                       tag="pmm", name="pmm")
                        tB = pmmp.tile([128, 2, 512], f32,
                                       tag="pmm", name="pmm")
                        for k in range(2):
                            ch = 2 * cg + k
                            csl = slice(ch * CHUNK, (ch + 1) * CHUNK)
                            nc.tensor.matmul(
                                tA[:, k, 0:CHUNK], lhsT_A,
                                wtb_c[0:EMB, csl],
                                start=True, stop=True,
                            )
                            nc.tensor.matmul(
                                tB[:, k, 0:CHUNK], lhsT_B,
                                wtb_c[EMB:128, csl],
                                start=True, stop=True,
                            )
                        keepalive(3)
                        nc.scalar.activation(
                            otA[:, 2 * cg:2 * cg + 2, :], tA[:, :, 0:CHUNK],
                            mybir.ActivationFunctionType.Copy,
                            scale=scale_A,
                        )
                        nc.vector.tensor_scalar_mul(
                            otB[:, 2 * cg:2 * cg + 2, :],
                            tB[:, :, 0:CHUNK], scale_B)
                        if cg == 1:
                            nc.sync.dma_start(
                                out_d[mA * 128:(mA + 1) * 128, 0:VS // 2],
                                otA[:, 0:NCHUNK // 2, :])
                            nc.sync.dma_start(
                                out_d[mB * 128:(mB + 1) * 128, 0:VS // 2],
                                otB[:, 0:NCHUNK // 2, :])
                        elif cg == 3:
                            nc.sync.dma_start(
                                out_d[mA * 128:(mA + 1) * 128, VS // 2:VS],
                                otA[:, NCHUNK // 2:NCHUNK, :])
                            nc.sync.dma_start(
                                out_d[mB * 128:(mB + 1) * 128, VS // 2:VS],
                                otB[:, NCHUNK // 2:NCHUNK, :])

            LEAD = 1
            seg_of = {}
            for Q in range(min(LEAD, NQ)):
                head(Q)
                seg_of[Q] = state["seg_c"]
            for Q in range(NQ):
                if Q + LEAD < NQ:
                    head(Q + LEAD)
                    seg_of[Q + LEAD] = state["seg_c"]
                proj(Q, seg_of.pop(Q))

    nc.compile()
    return nc


def _get_prog(compute: str, out_dt: str):
    key = (compute, out_dt)
    if key not in _prog_cache:
        _prog_cache[key] = _build(compute, out_dt)
    return _prog_cache[key]


def _make_in_maps(emb_table, W, b, x):
    emb_table = np.ascontiguousarray(np.asarray(emb_table, dtype=np.float32))
    W = np.asarray(W, dtype=np.float32)
    x = np.asarray(x).astype(np.int64).reshape(B, SEQ)

    # idx layout: token m*128 + p -> idx[p, m]
    wrapped = np.ascontiguousarray(
        x.reshape(-1).reshape(MTILES, 128).T.astype(np.int32)
    )

    i = np.arange(128)[:, None]
    mb = np.arange(BLK)[None, :]
    recip = (1.0 / (mb * 128 + i + 1)).astype(np.float32)

    in_maps = []
    for c in range(NCORES):
        wtb = np.empty((128, VS), dtype=np.float32)
        wtb[0:EMB] = W[c * VS:(c + 1) * VS, :].T
        wtb[EMB:128] = wtb[0:EMB]
        in_maps.append({
            "emb": emb_table,
            "idx": wrapped,
            "wtb": np.ascontiguousarray(wtb),
            "recip": recip,
        })
    return in_maps


def kernel(emb_table, W, b, x, trace=False):
    from concourse.bass_utils import run_bass_kernel_spmd

    nc = _get_prog(COMPUTE, OUT_DT)
    in_maps = _make_in_maps(emb_table, W, b, x)
    res = run_bass_kernel_spmd(
        nc, in_maps, core_ids=list(range(NCORES)), trace=trace,
    )

    b32 = np.asarray(b, dtype=np.float32)
    out = np.empty((TOK, VOCAB), dtype=np.float32)
    for c in range(NCORES):
        sl = slice(c * VS, (c + 1) * VS)
        np.add(np.asarray(res.results[c]["out"], dtype=np.float32),
               b32[sl][None, :], out=out[:, sl])
    out = out.reshape(B, SEQ, VOCAB)
    if trace:
        return out, res
    return out
